# revision 25
# baseline (speedup 1.0000x reference)
"""Sigmoid-attention block kernel for trn2 (one NeuronCore, SPMD over 8) — v5.

Host pre-folds the V projection (v_proj = value @ Wv.T + bv, computed in
float64) so the device does only:

  qT [H, SLAB]   = WqT.T-blocks @ queryT + bq       (bf16 matmuls)
  kT [H, N]      = WkT.T-blocks @ keyT + bk         (streamed chunks, bf16)
  attnT [N, SLAB] = sigmoid(kT.T @ qT)              (bf16 MMs, fp32 PSUM)
  outT [H, SLAB]  = sum_j v_proj[j].T-blocks @ attnT[j]   (bf16 MMs)

All DRAM inputs arrive in bf16 (host-converted), so there are no on-chip
casts and no value+delta adds; DVE only does the four bias adds per kT/qT
chunk. No epilogue matmul: outT is the PSUM accumulator content. The PE
stream is back-to-back bf16 N=512 matmuls (~216 ns each); the head starts
the kT chunk-0 projection as soon as its DMA lands, which doubles as HAM
warmup.
"""
from contextlib import ExitStack

import concourse.bass as bass
import concourse.mybir as mybir
import concourse.tile as tile
from concourse import bacc

F32 = mybir.dt.float32
BF16 = mybir.dt.bfloat16
AF = mybir.ActivationFunctionType


def _build_attn_kernel(SLAB=1024, N=8192, H=256):
    assert H == 256
    NJ = N // 128            # 64 j-blocks (rows of attnT)
    ICW = 512                # i-chunk width
    IC = SLAB // ICW         # 2
    KCW = 512                # key-chunk width = 4 j-blocks
    NKC = N // KCW           # 16
    JPC = KCW // 128         # 4
    HB = H // 128            # 2

    nc = bacc.Bacc()
    queryT = nc.dram_tensor("queryT", [H, SLAB], BF16, kind="ExternalInput")
    keyT = nc.dram_tensor("keyT", [H, N], BF16, kind="ExternalInput")
    valP = nc.dram_tensor("valP", [N, H], BF16, kind="ExternalInput")
    # host-packed: Wk.T and Wq.T as [128, 2*H] (hpb blocks side by side)
    WkP = nc.dram_tensor("WkP", [128, 2 * H], BF16, kind="ExternalInput")
    WqP = nc.dram_tensor("WqP", [128, 2 * H], BF16, kind="ExternalInput")
    # host-packed biases: cols = [bk0, bk1, bq0, bq1] per 128-partition block
    bP = nc.dram_tensor("bP", [128, 4], F32, kind="ExternalInput")
    outd = nc.dram_tensor("outT", [H, SLAB], BF16, kind="ExternalOutput")

    with tile.TileContext(nc) as tc, ExitStack() as ctx:
        cpool = ctx.enter_context(tc.tile_pool(name="const", bufs=1))
        psW = ctx.enter_context(tc.tile_pool(name="psW", bufs=4, space="PSUM"))
        psO = ctx.enter_context(tc.tile_pool(name="psO", bufs=1, space="PSUM"))
        big = ctx.enter_context(tc.tile_pool(name="big", bufs=1))
        krot = ctx.enter_context(tc.tile_pool(name="krot", bufs=2))
        valp = ctx.enter_context(tc.tile_pool(name="valp", bufs=10))
        attnp = ctx.enter_context(tc.tile_pool(name="attnp", bufs=3))
        outp = ctx.enter_context(tc.tile_pool(name="outp", bufs=1))

        # ---- head loads spread over three rings: the vector ring's queue
        # opens earliest, so it carries the critical-path wk + kchunk0 ----
        wk_p = cpool.tile([128, 2 * H], BF16, tag="wkp", name="wkp")
        nc.gpsimd.dma_start(wk_p[:], WkP[:, :])
        b_p = cpool.tile([128, 4], F32, tag="bp", name="bp")
        nc.scalar.dma_start(b_p[:], bP[:, :])
        wq_p = cpool.tile([128, 2 * H], BF16, tag="wqp", name="wqp")
        nc.scalar.dma_start(wq_p[:], WqP[:, :])


        kT_sb = [big.tile([128, N], BF16, tag=f"kT{hb}", name=f"kT{hb}")
                 for hb in range(HB)]
        qT_sb = [big.tile([128, SLAB], BF16, tag=f"qT{hb}", name=f"qT{hb}")
                 for hb in range(HB)]

        # HAM warmup: a few bf16 matmuls on a memset tile keep the PE busy
        # while the head DMAs land, so the first real matmuls run at 2.4 GHz.
        # DVE's queue opens ~1.3 us before GpSimd's, so memset there.
        wu = cpool.tile([128, 512], BF16, tag="wu", name="wu")
        nc.vector.memset(wu[:], 0.0)
        for _ in range(10):
            pw = psW.tile([128, 512], F32, tag="ps", name="pw")
            nc.tensor.matmul(pw[:], wu[:, :128], wu[:], start=True, stop=True)

        # ---- data DMAs on the sync ring (program order = priority) ----
        kch = {}

        def emit_kchunk_dma(c, ring=nc.sync):
            tiles = []
            for hb in range(HB):
                t = krot.tile([128, KCW], BF16, tag=f"kch{hb}",
                              name=f"kch{hb}", bufs=2)
                ring.dma_start(
                    t[:], keyT[hb * 128:(hb + 1) * 128, c * KCW:(c + 1) * KCW]
                )
                tiles.append(t)
            kch[c] = tiles

        emit_kchunk_dma(0, ring=nc.gpsimd)
        emit_kchunk_dma(1)
        qu = []
        for hpb in range(HB):
            t = big.tile([128, SLAB], BF16, tag=f"qu{hpb}", name=f"qu{hpb}")
            nc.scalar.dma_start(t[:], queryT[hpb * 128:(hpb + 1) * 128, :])
            qu.append(t)

        val_t = [None] * NJ

        def emit_val_dma(j):
            t = valp.tile([128, H], BF16, tag="val")
            nc.sync.dma_start(t[:], valP[j * 128:(j + 1) * 128, :])
            val_t[j] = t

        for j in range(6):
            emit_val_dma(j)

        # ---- compute emitters ----
        def emit_kchunk_mm(c):
            tiles = kch.pop(c)
            for hb in range(HB):
                pk = psW.tile([128, 512], F32, tag="ps")
                for hpb in range(HB):
                    nc.tensor.matmul(
                        pk[:],
                        wk_p[:, hpb * H + hb * 128:hpb * H + (hb + 1) * 128],
                        tiles[hpb][:],
                        start=(hpb == 0),
                        stop=(hpb == HB - 1),
                    )
                nc.vector.tensor_scalar_add(
                    kT_sb[hb][:, c * KCW:(c + 1) * KCW], pk[:],
                    b_p[:, hb:hb + 1],
                )

        def emit_qT():
            for hb in range(HB):
                for ic in range(IC):
                    pq = psW.tile([128, 512], F32, tag="ps")
                    for hpb in range(HB):
                        nc.tensor.matmul(
                            pq[:, :ICW],
                            wq_p[:, hpb * H + hb * 128:hpb * H + (hb + 1) * 128],
                            qu[hpb][:, ic * ICW:(ic + 1) * ICW],
                            start=(hpb == 0),
                            stop=(hpb == HB - 1),
                        )
                    nc.vector.tensor_scalar_add(
                        qT_sb[hb][:, ic * ICW:(ic + 1) * ICW], pq[:, :ICW],
                        b_p[:, 2 + hb:3 + hb],
                    )

        at_tiles = [None] * NJ

        def emit_logits(j):
            at = attnp.tile([128, SLAB], BF16, tag="at")
            for ic in range(IC):
                pl = psW.tile([128, 512], F32, tag="ps")
                for hb in range(HB):
                    nc.tensor.matmul(
                        pl[:],
                        kT_sb[hb][:, j * 128:(j + 1) * 128],
                        qT_sb[hb][:, ic * ICW:(ic + 1) * ICW],
                        start=(hb == 0),
                        stop=(hb == HB - 1),
                    )
                nc.scalar.activation(at[:, ic * ICW:(ic + 1) * ICW], pl[:],
                                     AF.Sigmoid)
            at_tiles[j] = at

        ps_o = [psO.tile([128, SLAB], F32, tag=f"po{hb}", name=f"po{hb}")
                for hb in range(HB)]

        def emit_oacc(j):
            at = at_tiles[j]
            for hb in range(HB):
                for ic in range(IC):
                    nc.tensor.matmul(
                        ps_o[hb][:, ic * ICW:(ic + 1) * ICW],
                        val_t[j][:, hb * 128:(hb + 1) * 128],
                        at[:, ic * ICW:(ic + 1) * ICW],
                        start=(j == 0),
                        stop=(j == NJ - 1),
                    )
            at_tiles[j] = None
            val_t[j] = None

        # ---- schedule ----
        emit_kchunk_mm(0)
        # hole-fillers: kchunk 1 / queryT land ~1 us after chunk 0
        for _ in range(2):
            pw = psW.tile([128, 512], F32, tag="ps", name="pw")
            nc.tensor.matmul(pw[:], wu[:, :128], wu[:], start=True, stop=True)
        emit_kchunk_mm(1)
        emit_qT()
        emit_kchunk_dma(2)
        emit_kchunk_dma(3)
        j = 0
        for c in range(NKC):
            # logits/oacc for the j-blocks of chunk c (projected 2 iters ago)
            for t in range(JPC):
                if j + 6 < NJ:
                    emit_val_dma(j + 6)
                emit_logits(j)
                if j >= 1:
                    emit_oacc(j - 1)
                j += 1
            if c + 2 < NKC:
                emit_kchunk_mm(c + 2)
            if c + 4 < NKC:
                emit_kchunk_dma(c + 4)
        emit_oacc(NJ - 1)

        # ---- tail: copy PSUM accumulators out (bf16), DVE/ACT copies in
        # parallel, DMAs split across both rings
        rings = [nc.sync, nc.scalar]
        for hb in range(HB):
            for ic in range(IC):
                o = outp.tile([128, ICW], BF16, tag=f"o{hb}{ic}",
                              name=f"o{hb}{ic}")
                nc.vector.tensor_copy(o[:], ps_o[hb][:, ic * ICW:(ic + 1) * ICW])
                rings[(2 * hb + ic) % 2].dma_start(
                    outd[hb * 128:(hb + 1) * 128, ic * ICW:(ic + 1) * ICW],
                    o[:],
                )

    nc.finalize()
    return nc


import numpy as np
import ml_dtypes
from concourse.bass_utils import run_bass_kernel_spmd

BF16_NP = ml_dtypes.bfloat16

N_CORES = 8
N_FULL = 8192
H_FULL = 256
SLAB_FULL = N_FULL // N_CORES

_NC = None


def _get_nc():
    global _NC
    if _NC is None:
        _NC = _build_attn_kernel(SLAB=SLAB_FULL, N=N_FULL, H=H_FULL)
    return _NC


def _in_maps(inputs):
    full = {k: np.asarray(v, dtype=np.float32) for k, v in inputs.items()}
    # fold the V projection on the host (float64): v_proj = value @ Wv.T + bv
    vP = (full["value"].astype(np.float64) @ full["Wv"].astype(np.float64).T
          + full["bv"].astype(np.float64))
    queryT = np.ascontiguousarray(full["query"].T).astype(BF16_NP)  # [H, N]
    # pack W.T [2*128, H] as [128, 2*H] (hpb blocks side by side), and the
    # four per-partition bias columns as one [128, 4] f32 array
    WkT = full["Wk"].T.astype(BF16_NP)
    WqT = full["Wq"].T.astype(BF16_NP)
    bP = np.stack([full["bk"][:128], full["bk"][128:],
                   full["bq"][:128], full["bq"][128:]], axis=1)
    shared = {
        "keyT": np.ascontiguousarray(full["key"].T).astype(BF16_NP),
        "valP": np.ascontiguousarray(vP).astype(BF16_NP),
        "WkP": np.ascontiguousarray(
            np.concatenate([WkT[:128], WkT[128:]], axis=1)),
        "WqP": np.ascontiguousarray(
            np.concatenate([WqT[:128], WqT[128:]], axis=1)),
        "bP": np.ascontiguousarray(bP.astype(np.float32)),
    }
    maps = []
    for c in range(N_CORES):
        m = dict(shared)
        m["queryT"] = np.ascontiguousarray(
            queryT[:, c * SLAB_FULL:(c + 1) * SLAB_FULL]
        )
        maps.append(m)
    return maps


def kernel(**inputs) -> np.ndarray:
    nc = _get_nc()
    res = run_bass_kernel_spmd(nc, _in_maps(inputs), list(range(N_CORES)))
    return np.ascontiguousarray(np.concatenate(
        [np.asarray(res.results[c]["outT"]).astype(np.float32).T
         for c in range(N_CORES)],
        axis=0,
    )).astype(np.float32)


# revision 29
# speedup vs baseline: 1.0477x; 1.0477x over previous
"""Sigmoid-attention block kernel for trn2 (one NeuronCore, SPMD over 8) — v5.

Host pre-folds the V projection (v_proj = value @ Wv.T + bv, computed in
float64) so the device does only:

  qT [H, SLAB]   = WqT.T-blocks @ queryT + bq       (bf16 matmuls)
  kT [H, N]      = WkT.T-blocks @ keyT + bk         (streamed chunks, bf16)
  attnT [N, SLAB] = sigmoid(kT.T @ qT)              (bf16 MMs, fp32 PSUM)
  outT [H, SLAB]  = sum_j v_proj[j].T-blocks @ attnT[j]   (bf16 MMs)

All DRAM inputs arrive in bf16 (host-converted), so there are no on-chip
casts and no value+delta adds; DVE only does the four bias adds per kT/qT
chunk. No epilogue matmul: outT is the PSUM accumulator content. The PE
stream is back-to-back bf16 N=512 matmuls (~216 ns each); the head starts
the kT chunk-0 projection as soon as its DMA lands, which doubles as HAM
warmup.
"""
from contextlib import ExitStack

import concourse.bass as bass
import concourse.mybir as mybir
import concourse.tile as tile
from concourse import bacc

F32 = mybir.dt.float32
BF16 = mybir.dt.bfloat16
AF = mybir.ActivationFunctionType


def _build_attn_kernel(SLAB=1024, N=8192, H=256):
    assert H == 256
    NJ = N // 128            # 64 j-blocks (rows of attnT)
    ICW = 512                # i-chunk width
    IC = SLAB // ICW         # 2
    KCW = 512                # key-chunk width = 4 j-blocks
    NKC = N // KCW           # 16
    JPC = KCW // 128         # 4
    HB = H // 128            # 2

    nc = bacc.Bacc()
    queryT = nc.dram_tensor("queryT", [H, SLAB], BF16, kind="ExternalInput")
    keyT = nc.dram_tensor("keyT", [H, N], BF16, kind="ExternalInput")
    valP = nc.dram_tensor("valP", [N, H], BF16, kind="ExternalInput")
    # host-fused logit weight M = Wq.T @ Wk, packed M.T as [128, 2*H]
    MP = nc.dram_tensor("MP", [128, 2 * H], BF16, kind="ExternalInput")
    # host-packed k' bias (Wq.T @ bk): cols = [kb0, kb1] per partition block
    bP = nc.dram_tensor("bP", [128, 2], F32, kind="ExternalInput")
    # host-computed per-row logit offset t = key @ (Wk.T bq) + bq.bk, [128, NJ]
    tT = nc.dram_tensor("tT", [128, N // 128], F32, kind="ExternalInput")
    outd = nc.dram_tensor("outT", [H, SLAB], BF16, kind="ExternalOutput")

    with tile.TileContext(nc) as tc, ExitStack() as ctx:
        cpool = ctx.enter_context(tc.tile_pool(name="const", bufs=1))
        psW = ctx.enter_context(tc.tile_pool(name="psW", bufs=4, space="PSUM"))
        psO = ctx.enter_context(tc.tile_pool(name="psO", bufs=1, space="PSUM"))
        big = ctx.enter_context(tc.tile_pool(name="big", bufs=1))
        krot = ctx.enter_context(tc.tile_pool(name="krot", bufs=2))
        valp = ctx.enter_context(tc.tile_pool(name="valp", bufs=10))
        attnp = ctx.enter_context(tc.tile_pool(name="attnp", bufs=3))
        outp = ctx.enter_context(tc.tile_pool(name="outp", bufs=1))

        # ---- weight/bias/t loads on the scalar ring ----
        wk_p = cpool.tile([128, 2 * H], BF16, tag="wkp", name="wkp")
        nc.scalar.dma_start(wk_p[:], MP[:, :])
        b_p = cpool.tile([128, 2], F32, tag="bp", name="bp")
        nc.scalar.dma_start(b_p[:], bP[:, :])
        t_sb = cpool.tile([128, N // 128], F32, tag="tT", name="tT")
        nc.scalar.dma_start(t_sb[:], tT[:, :])

        kT_sb = [big.tile([128, N], BF16, tag=f"kT{hb}", name=f"kT{hb}")
                 for hb in range(HB)]
        # HAM warmup: a few bf16 matmuls on a memset tile keep the PE busy
        # while the head DMAs land, so the first real matmuls run at 2.4 GHz
        wu = cpool.tile([128, 512], BF16, tag="wu", name="wu")
        nc.gpsimd.memset(wu[:], 0.0)
        for _ in range(10):
            pw = psW.tile([128, 512], F32, tag="ps", name="pw")
            nc.tensor.matmul(pw[:], wu[:, :128], wu[:], start=True, stop=True)

        # ---- data DMAs on the sync ring (program order = priority) ----
        kch = {}

        def emit_kchunk_dma(c, ring=nc.sync):
            tiles = []
            for hb in range(HB):
                t = krot.tile([128, KCW], BF16, tag=f"kch{hb}",
                              name=f"kch{hb}", bufs=2)
                ring.dma_start(
                    t[:], keyT[hb * 128:(hb + 1) * 128, c * KCW:(c + 1) * KCW]
                )
                tiles.append(t)
            kch[c] = tiles

        emit_kchunk_dma(0)
        emit_kchunk_dma(1)
        qu = []
        for hpb in range(HB):
            t = big.tile([128, SLAB], BF16, tag=f"qu{hpb}", name=f"qu{hpb}")
            nc.scalar.dma_start(t[:], queryT[hpb * 128:(hpb + 1) * 128, :])
            qu.append(t)

        val_t = [None] * NJ

        def emit_val_dma(j):
            t = valp.tile([128, H], BF16, tag="val")
            nc.sync.dma_start(t[:], valP[j * 128:(j + 1) * 128, :])
            val_t[j] = t

        for j in range(6):
            emit_val_dma(j)

        # ---- compute emitters ----
        def emit_kchunk_mm(c):
            tiles = kch.pop(c)
            for hb in range(HB):
                pk = psW.tile([128, 512], F32, tag="ps")
                for hpb in range(HB):
                    nc.tensor.matmul(
                        pk[:],
                        wk_p[:, hpb * H + hb * 128:hpb * H + (hb + 1) * 128],
                        tiles[hpb][:],
                        start=(hpb == 0),
                        stop=(hpb == HB - 1),
                    )
                nc.vector.tensor_scalar_add(
                    kT_sb[hb][:, c * KCW:(c + 1) * KCW], pk[:],
                    b_p[:, hb:hb + 1],
                )

        at_tiles = [None] * NJ

        def emit_logits(j):
            at = attnp.tile([128, SLAB], BF16, tag="at")
            for ic in range(IC):
                pl = psW.tile([128, 512], F32, tag="ps")
                for hb in range(HB):
                    nc.tensor.matmul(
                        pl[:],
                        kT_sb[hb][:, j * 128:(j + 1) * 128],
                        qu[hb][:, ic * ICW:(ic + 1) * ICW],
                        start=(hb == 0),
                        stop=(hb == HB - 1),
                    )
                nc.scalar.activation(at[:, ic * ICW:(ic + 1) * ICW], pl[:],
                                     AF.Sigmoid, bias=t_sb[:, j:j + 1])
            at_tiles[j] = at

        ps_o = [psO.tile([128, SLAB], F32, tag=f"po{hb}", name=f"po{hb}")
                for hb in range(HB)]

        def emit_oacc(j):
            at = at_tiles[j]
            for hb in range(HB):
                for ic in range(IC):
                    nc.tensor.matmul(
                        ps_o[hb][:, ic * ICW:(ic + 1) * ICW],
                        val_t[j][:, hb * 128:(hb + 1) * 128],
                        at[:, ic * ICW:(ic + 1) * ICW],
                        start=(j == 0),
                        stop=(j == NJ - 1),
                    )
            at_tiles[j] = None
            val_t[j] = None

        # ---- schedule ----
        emit_kchunk_mm(0)
        # hole-fillers: kchunk 1 / queryT land ~1 us after chunk 0
        for _ in range(2):
            pw = psW.tile([128, 512], F32, tag="ps", name="pw")
            nc.tensor.matmul(pw[:], wu[:, :128], wu[:], start=True, stop=True)
        emit_kchunk_mm(1)
        emit_kchunk_dma(2)
        emit_kchunk_dma(3)
        j = 0
        for c in range(NKC):
            # logits/oacc for the j-blocks of chunk c (projected 2 iters ago)
            for t in range(JPC):
                if j + 6 < NJ:
                    emit_val_dma(j + 6)
                emit_logits(j)
                if j >= 1:
                    emit_oacc(j - 1)
                j += 1
            if c + 2 < NKC:
                emit_kchunk_mm(c + 2)
            if c + 4 < NKC:
                emit_kchunk_dma(c + 4)
        emit_oacc(NJ - 1)

        # ---- tail: copy PSUM accumulators out (bf16), DVE/ACT copies in
        # parallel, DMAs split across both rings
        rings = [nc.sync, nc.scalar]
        for hb in range(HB):
            for ic in range(IC):
                o = outp.tile([128, ICW], BF16, tag=f"o{hb}{ic}",
                              name=f"o{hb}{ic}")
                nc.vector.tensor_copy(o[:], ps_o[hb][:, ic * ICW:(ic + 1) * ICW])
                rings[(2 * hb + ic) % 2].dma_start(
                    outd[hb * 128:(hb + 1) * 128, ic * ICW:(ic + 1) * ICW],
                    o[:],
                )

    nc.finalize()
    return nc


import numpy as np
import ml_dtypes
from concourse.bass_utils import run_bass_kernel_spmd

BF16_NP = ml_dtypes.bfloat16

N_CORES = 8
N_FULL = 8192
H_FULL = 256
SLAB_FULL = N_FULL // N_CORES

_NC = None


def _get_nc():
    global _NC
    if _NC is None:
        _NC = _build_attn_kernel(SLAB=SLAB_FULL, N=N_FULL, H=H_FULL)
    return _NC


def _in_maps(inputs):
    full = {k: np.asarray(v, dtype=np.float32) for k, v in inputs.items()}
    # fold the V projection on the host (float64): v_proj = value @ Wv.T + bv
    vP = (full["value"].astype(np.float64) @ full["Wv"].astype(np.float64).T
          + full["bv"].astype(np.float64))
    queryT = np.ascontiguousarray(full["query"].T).astype(BF16_NP)  # [H, N]
    # Fold both projections into the K side (float64 on host):
    #   logit_ij = k_j . q_i = key_j . (M key-side proj) + t_j
    #   k' = key @ M.T + kb,  M = Wq.T @ Wk,  kb = Wq.T @ bk
    #   t  = key @ (Wk.T @ bq) + bq . bk     (per-j sigmoid bias)
    Wq64 = full["Wq"].astype(np.float64)
    Wk64 = full["Wk"].astype(np.float64)
    bq64 = full["bq"].astype(np.float64)
    bk64 = full["bk"].astype(np.float64)
    M = Wq64.T @ Wk64                      # k' = M @ key_j (columns)
    MT = np.ascontiguousarray(M.T).astype(BF16_NP)   # lhsT layout
    kb = (Wq64.T @ bk64).astype(np.float32)
    t = (full["key"].astype(np.float64) @ (Wk64.T @ bq64)
         + bq64 @ bk64).astype(np.float32)           # [N]
    NJ = N_FULL // 128
    shared = {
        "keyT": np.ascontiguousarray(full["key"].T).astype(BF16_NP),
        "valP": np.ascontiguousarray(vP).astype(BF16_NP),
        "MP": np.ascontiguousarray(
            np.concatenate([MT[:128], MT[128:]], axis=1)),
        "bP": np.ascontiguousarray(
            np.stack([kb[:128], kb[128:]], axis=1)),
        "tT": np.ascontiguousarray(t.reshape(NJ, 128).T),
    }
    maps = []
    for c in range(N_CORES):
        m = dict(shared)
        m["queryT"] = np.ascontiguousarray(
            queryT[:, c * SLAB_FULL:(c + 1) * SLAB_FULL]
        )
        maps.append(m)
    return maps


def kernel(**inputs) -> np.ndarray:
    nc = _get_nc()
    res = run_bass_kernel_spmd(nc, _in_maps(inputs), list(range(N_CORES)))
    return np.ascontiguousarray(np.concatenate(
        [np.asarray(res.results[c]["outT"]).astype(np.float32).T
         for c in range(N_CORES)],
        axis=0,
    )).astype(np.float32)

